# revision 1
# baseline (speedup 1.0000x reference)
"""BiLSTM-CRF Trainium2 kernel.

Full-input contract: kernel(**inputs) takes the unsharded numpy inputs and
returns the full [64, 512, 32, 32] float32 output. Internally shards the
batch (64) across 8 NeuronCores (8 sentences per core), runs a Bass kernel
SPMD, and concatenates the per-core outputs.

Per-core device pipeline:
  1. Embedding gather (indirect DMA) for fwd token order and reversed order.
  2. PE-transpose gathered rows into xT [E=128, L*B] column-major-by-(t,b).
  3. JIT precompute of P = Wih @ x (+bias) in windows of 32 timesteps.
  4. 512-step LSTM scan for both directions (independent chains, interleaved).
     Gate layout: gates.T [128 partitions = gate-dim-in-chunk, 8 chunks * B].
     Chunk order [i0 i1 f0 f1 o0 o1 g0 g1] so sigmoid(i,f,o) is one ACT op.
  5. Emission matmul emisT[33, t*b] = W_linT_aug.T @ h_cat.T (row 32 == 1).
  6. CRF expand: crf[128 rows=(t,b), 1024=(i,j)] = emisT_slice.T @ Jsel_aug
     where Jsel_aug row 32 carries transition+b_lin -> broadcast add fused
     into the same accumulation. Copy PSUM->SBUF, DMA out.
"""

import numpy as np

VOCAB, EMB, HID, OUT = 30000, 128, 256, 32
B, L = 64, 512
NCORES = 8
BC = B // NCORES  # batch per core = 8

# Scan-path precision: bf16 weights/x/P/h (fp32 PSUM accumulation).
# Halves PE weight-load time (FWL) and enables fast DVE modes.
SCAN_BF16 = True


def _host_prep(inputs, L_eff=L):
    """Prepare per-core in_maps (list of dicts) from full inputs."""
    import concourse.mybir as mybir  # noqa

    sents = np.asarray(inputs["sents_tensor"]).astype(np.int32)  # [B, L]
    emb = np.asarray(inputs["embedding"]).astype(np.float32)  # [V, E]

    # gate permutation: torch order i,f,g,o -> ours i,f,o,g
    perm = np.concatenate([np.arange(0, 512), np.arange(768, 1024),
                           np.arange(512, 768)])

    def mk_wT(Wih, Whh, bih, bhh):
        Wih = np.asarray(Wih, np.float32)[perm]  # [1024, 128]
        Whh = np.asarray(Whh, np.float32)[perm]  # [1024, 256]
        wT = np.stack([
            np.ascontiguousarray(Wih.T),              # [128, 1024]
            np.ascontiguousarray(Whh[:, :128].T),     # [128, 1024]
            np.ascontiguousarray(Whh[:, 128:].T),     # [128, 1024]
        ])  # [3, 128, 1024]
        b = (np.asarray(bih, np.float32) + np.asarray(bhh, np.float32))[perm]
        b_sb = np.ascontiguousarray(b.reshape(8, 128).T)  # [128, 8] col=chunk
        return wT, b_sb

    wT_f, b_f = mk_wT(inputs["Wih_f"], inputs["Whh_f"], inputs["bih_f"],
                      inputs["bhh_f"])
    wT_b, b_b = mk_wT(inputs["Wih_b"], inputs["Whh_b"], inputs["bih_b"],
                      inputs["bhh_b"])
    wT = np.stack([wT_f, wT_b])          # [2, 3, 128, 1024]
    if SCAN_BF16:
        import ml_dtypes
        wT = wT.astype(ml_dtypes.bfloat16)
    bias = np.stack([b_f, b_b], axis=-1)  # [128, 8, 2] col = c + 8*d? keep sep
    bias = np.ascontiguousarray(
        np.concatenate([b_f, b_b], axis=1))  # [128, 16]: cols 0:8 fwd, 8:16 bwd

    W_lin = np.asarray(inputs["W_lin"], np.float32)      # [32, 2H]
    b_lin = np.asarray(inputs["b_lin"], np.float32)      # [32]
    trans = np.asarray(inputs["transition"], np.float32)  # [32, 32]

    WlinT = np.ascontiguousarray(W_lin.T)  # [512, 32]
    WlinT_aug = np.zeros([4, 128, 33], np.float32)
    for kt in range(4):
        WlinT_aug[kt, :, :32] = WlinT[kt * 128:(kt + 1) * 128]
    if SCAN_BF16:
        import ml_dtypes
        WlinT_aug = WlinT_aug.astype(ml_dtypes.bfloat16)

    Jsel_aug = np.zeros([33, 1024], np.float32)
    for i in range(32):
        for j in range(32):
            Jsel_aug[j, i * 32 + j] = 1.0
    Jsel_aug[32] = (trans + b_lin[None, :]).reshape(-1)

    emis_bias = np.zeros([33, 1], np.float32)
    emis_bias[32, 0] = 1.0  # makes emisT row 32 == 1 after ACT copy w/ bias

    in_maps = []
    for c in range(NCORES):
        s = sents[c * BC:(c + 1) * BC, :L_eff]  # [BC, L]
        sT = np.ascontiguousarray(s.T)          # [L, BC] token (t,b)
        sTr = np.ascontiguousarray(s[:, ::-1].T)  # reversed time order
        ntok = L_eff * BC
        ntiles = ntok // 128
        idx = np.ascontiguousarray(
            sT.reshape(ntiles, 128).T).astype(np.int32)   # [128, ntiles]
        idx_r = np.ascontiguousarray(
            sTr.reshape(ntiles, 128).T).astype(np.int32)  # [128, ntiles]
        # pack 4-byte-per-element constants into one [128, *] uint32 tensor:
        # idx (2*ntiles int32) then bias (16 fp32)
        idx_all = np.concatenate([idx, idx_r], axis=1)  # [128, 2*ntiles]
        wT_pm = np.ascontiguousarray(
            wT.transpose(2, 0, 1, 3).reshape(128, -1))     # [128, 6*1024]
        wl_pm = np.ascontiguousarray(
            WlinT_aug.transpose(1, 0, 2).reshape(128, -1))  # [128, 4*33]
        c4_parts = [idx_all.view(np.uint32), bias.view(np.uint32)]
        if SCAN_BF16:
            c2 = np.concatenate([wT_pm, wl_pm], axis=1)  # bf16 lane
        else:
            c2 = None
            c4_parts += [wT_pm.view(np.uint32), wl_pm.view(np.uint32)]
        c4 = np.concatenate(c4_parts, axis=1)
        # jsel + emis ones-row bias column
        cj = np.concatenate([Jsel_aug, emis_bias], axis=1)  # [33, 1025]
        m = {
            "c4": np.ascontiguousarray(c4),
            "emb": emb,
            "cj": np.ascontiguousarray(cj),
        }
        if c2 is not None:
            m["c2"] = np.ascontiguousarray(c2)
        in_maps.append(m)
    return in_maps


def build_nc(L_eff=L, reps=1, timing=False):
    """Build the Bass program (identical for every core).

    reps>1 repeats the compute body (scan+emission) N times; timing=True
    swaps the big external tensors (embedding in, crf out) for internal
    DRAM so benchmark calls transfer almost nothing over the axon tunnel.
    """
    import concourse.bass as bass
    import concourse.mybir as mybir
    import concourse.tile as tile
    from concourse.bacc import Bacc
    from concourse.masks import make_identity

    dt = mybir.dt
    AF = mybir.ActivationFunctionType
    OP = mybir.AluOpType

    NTOK = L_eff * BC          # tokens per direction
    NTILE = NTOK // 128        # gather tiles per direction
    WIN = 32 if L_eff >= 32 else L_eff  # steps per P window
    NWIN = L_eff // WIN
    NW = WIN * BC              # P-window token count (cols per chunk)

    nc = Bacc()

    N4 = 2 * NTILE + 16 + (0 if SCAN_BF16 else 6 * 1024 + 4 * 33)
    d_c4 = nc.declare_dram_parameter("c4", [128, N4], dt.uint32, False)
    if timing:
        d_emb = nc.dram_tensor("embt", [VOCAB, EMB], dt.float32)
    else:
        d_emb = nc.declare_dram_parameter("emb", [VOCAB, EMB], dt.float32,
                                          False)
    d_cj = nc.declare_dram_parameter("cj", [33, 1025], dt.float32r, False)
    if SCAN_BF16:
        d_c2 = nc.declare_dram_parameter("c2", [128, 6 * 1024 + 4 * 33],
                                         dt.bfloat16, False)
    if timing:
        d_out = nc.dram_tensor("outt", [BC, L_eff, 1024], dt.float32)
        d_out_ext = nc.declare_dram_parameter("out", [1, 16], dt.float32,
                                              isOutput=True)
    else:
        d_out = nc.declare_dram_parameter("out", [BC, L_eff, 1024],
                                          dt.float32, isOutput=True)
        d_out_ext = None

    def r(ap):  # fp32 -> fp32r view for fast moving operands
        return ap.bitcast(dt.float32r)

    SDT = dt.bfloat16 if SCAN_BF16 else dt.float32  # scan-path dtype

    def s_r(ap):  # scan-path moving-operand view
        return ap if SCAN_BF16 else ap.bitcast(dt.float32r)

    with tile.TileContext(nc) as tc:
        with (
            tc.tile_pool(name="const", bufs=1) as const,
            tc.tile_pool(name="state", bufs=1) as state,
        ):
            # ---- constants / persistent tiles (3 DMA calls total, to keep
            # per-instruction semaphore-wait counts under the ISA limit) ----
            ident = const.tile([128, 128], dt.float32)
            make_identity(nc, ident[:])
            ident_s = const.tile([128, 128], SDT)
            nc.vector.tensor_copy(out=ident_s[:], in_=ident[:])
            c4_sb = const.tile([128, N4], dt.uint32)
            nc.sync.dma_start(out=c4_sb[:], in_=d_c4[:])
            idx_sb = c4_sb[:, 0:2 * NTILE].bitcast(dt.int32)
            bias_sb = c4_sb[:, 2 * NTILE:2 * NTILE + 16].bitcast(dt.float32)
            if SCAN_BF16:
                c2_sb = const.tile([128, 6 * 1024 + 4 * 33], dt.bfloat16)
                nc.sync.dma_start(out=c2_sb[:], in_=d_c2[:])
                wT_sb = c2_sb[:, 0:6 * 1024]
                wlin_sb = c2_sb[:, 6 * 1024:]
            else:
                o4 = 2 * NTILE + 16
                wT_sb = c4_sb[:, o4:o4 + 6 * 1024].bitcast(dt.float32)
                wlin_sb = c4_sb[:, o4 + 6 * 1024:].bitcast(dt.float32)
            cj_sb = const.tile([33, 1025], dt.float32r)
            nc.sync.dma_start(out=cj_sb[:], in_=d_cj[:])
            jsel_sb = cj_sb[:, 0:1024]
            ebias_sb = cj_sb[:, 1024:1025].bitcast(dt.float32)

            def wTd(d, kt):  # [128, 1024] weight K-tile
                off = (d * 3 + kt) * 1024
                return wT_sb[:, off:off + 1024]

            # persistent big buffers
            xT = state.tile([128, 2 * NTOK], SDT)   # cols: d*NTOK+(t,b)
            h_all = state.tile([128, 2 * L_eff * 16], SDT)
            zero16 = state.tile([128, 16], SDT)
            nc.vector.memset(zero16[:], 0.0)

            # ---- all pools stay open for the whole kernel: releasing a
            # pool lets later pools reuse its SBUF range, which creates
            # WAR deps on every DMA that wrote it (sem-wait-count blowup).
            with (
                tc.tile_pool(name="gat", bufs=4) as gat,
                tc.tile_pool(name="pwin", bufs=2) as pwin,
                tc.tile_pool(name="jit_ps", bufs=2, space="PSUM") as jit_ps,
                tc.tile_pool(name="gates_ps", bufs=3, space="PSUM") as gates_ps,
                tc.tile_pool(name="cpool", bufs=2) as cpool,
                tc.tile_pool(name="spool", bufs=3) as spool,
                tc.tile_pool(name="emis_ps", bufs=1, space="PSUM") as emis_ps,
                tc.tile_pool(name="emis_sb", bufs=2) as emis_sb_p,
                tc.tile_pool(name="crf_ps", bufs=2, space="PSUM") as crf_ps,
                tc.tile_pool(name="crf_sb", bufs=3) as crf_sb_p,
            ):
                # absorb the identity-ready (Pool) wait into a throwaway PE
                # transpose so real transposes carry only their gather wait
                # (matmul ISA sync-wait slots are scarce)
                pt0 = jit_ps.tile([128, NW], dt.float32, tag="jp")
                nc.tensor.transpose(out=pt0[:, 0:128], in_=ident[:],
                                    identity=ident[:])

                # ---- embedding gather + transpose to xT ----
                if timing:
                    # the timing build's embedding table is uninitialized
                    # DRAM; gathered garbage (denormals/NaN) would poison the
                    # whole scan with slow-path arithmetic. Zero xT instead —
                    # gathers are outside the repeated body and don't affect
                    # the marginal-time measurement.
                    nc.vector.memset(xT[:], 0.0)
                for g in range(0 if timing else 2 * NTILE):
                    gt = gat.tile([128, 128], dt.float32, tag="g")
                    nc.gpsimd.indirect_dma_start(
                        out=gt[:], out_offset=None, in_=d_emb[:],
                        in_offset=bass.IndirectOffsetOnAxis(
                            ap=idx_sb[:, g:g + 1], axis=0),
                    )
                    pt = jit_ps.tile([128, NW], dt.float32, tag="jp")
                    nc.tensor.transpose(out=pt[:, 0:128], in_=gt[:],
                                        identity=ident[:])
                    if g % 2 == 0:
                        nc.vector.tensor_copy(
                            out=xT[:, g * 128:(g + 1) * 128], in_=pt[:, 0:128])
                    else:
                        nc.scalar.copy(
                            out=xT[:, g * 128:(g + 1) * 128], in_=pt[:, 0:128])
                def jit_window(d, w):
                    """P window: [128, 8 chunks * NW], col = c*NW + t_l*8 + b."""
                    P = pwin.tile([128, 8 * NW], SDT, tag=f"P{d}")
                    for c in range(8):
                        ps = jit_ps.tile([128, NW], dt.float32, tag="jp")
                        nc.tensor.matmul(
                            out=ps[:],
                            lhsT=s_r(wTd(d, 0)[:, c * 128:(c + 1) * 128]),
                            rhs=s_r(xT[:, d * NTOK + w * NW:
                                       d * NTOK + (w + 1) * NW]),
                            start=True, stop=True)
                        # copy + per-partition bias add
                        if c % 2 == 0:
                            nc.scalar.activation(
                                out=P[:, c * NW:(c + 1) * NW], in_=ps[:],
                                func=AF.Identity,
                                bias=bias_sb[:, d * 8 + c:d * 8 + c + 1])
                        else:
                            nc.vector.tensor_scalar(
                                out=P[:, c * NW:(c + 1) * NW], in0=ps[:],
                                scalar1=bias_sb[:, d * 8 + c:d * 8 + c + 1],
                                scalar2=None, op0=OP.add)
                    return P

                def h_slot(d, t):
                    off = d * L_eff * 16 + t * 16
                    return h_all[:, off:off + 16]

                c_prev = [None, None]
                P_cur = [None, None]

                def scan_step(d, s):
                    """One LSTM step for direction d at step s.
                    fwd: t = s; bwd: t = L-1-s (h written at original t)."""
                    t = s if d == 0 else L_eff - 1 - s
                    w, s_l = divmod(s, WIN)
                    if s_l == 0:
                        P_cur[d] = jit_window(d, w)
                    P = P_cur[d]
                    hp = zero16[:] if s == 0 else h_slot(d, t + (1 if d else -1))
                    g_ps = gates_ps.tile([128, 64], dt.float32, tag="g")
                    # init PSUM with P_t via identity matmul (PE does the add;
                    # measured equal-or-better than a DVE add and shortens the
                    # cross-engine dependency chain)
                    nc.tensor.matmul(
                        out=g_ps[:], lhsT=ident_s[:],
                        rhs=P.rearrange("p (c n) -> p c n", c=8)
                             [:, :, s_l * 8:(s_l + 1) * 8],
                        start=True, stop=True)
                    for c in range(8):
                        for kt in (1, 2):
                            nc.tensor.matmul(
                                out=g_ps[:, c * 8:(c + 1) * 8],
                                lhsT=wTd(d, kt)[:, c * 128:(c + 1) * 128],
                                rhs=hp[:, (kt - 1) * 8:kt * 8],
                                start=False, stop=False,
                                skip_group_check=True)
                    sg = spool.tile([128, 64], dt.float32, tag="s")
                    # sigmoid over i,f,o (cols 0:48); tanh over g (48:64)
                    nc.scalar.activation(out=sg[:, 0:48], in_=g_ps[:, 0:48],
                                         func=AF.Sigmoid)
                    nc.scalar.activation(out=sg[:, 48:64], in_=g_ps[:, 48:64],
                                         func=AF.Tanh)
                    c_new = cpool.tile([128, 16], dt.float32, tag="c")
                    # c_new = sig_i * tanh_g
                    nc.vector.tensor_tensor(out=c_new[:], in0=sg[:, 0:16],
                                            in1=sg[:, 48:64], op=OP.mult)
                    if s > 0:
                        # tmp = sig_f * c_prev  (reuse sg cols 16:32 as scratch)
                        nc.vector.tensor_tensor(out=sg[:, 16:32],
                                                in0=sg[:, 16:32],
                                                in1=c_prev[d][:], op=OP.mult)
                        nc.vector.tensor_tensor(out=c_new[:], in0=c_new[:],
                                                in1=sg[:, 16:32], op=OP.add)
                    c_prev[d] = c_new
                    # tanh(c) -> reuse sg cols 48:64
                    nc.scalar.activation(out=sg[:, 48:64], in_=c_new[:],
                                         func=AF.Tanh)
                    nc.vector.tensor_tensor(out=h_slot(d, t), in0=sg[:, 32:48],
                                            in1=sg[:, 48:64], op=OP.mult)

                def emit_block(k):
                    """Emission + CRF + DMA for t block [k*TB, (k+1)*TB).

                    emisT cols are (b, t)-ordered so each CRF row-tile's 128
                    partitions = (4 b's x TBLK t's) map 1:1 onto a plain
                    d_out[b0:b0+4, t0:t0+TBLK, :] DMA slice.
                    """
                    t0 = k * TBLK
                    n = TBLK * BC  # 256 cols
                    eps = emis_ps.tile([33, n], dt.float32, tag="e")
                    for kt in range(4):
                        d = kt // 2
                        c = kt % 2
                        rhs = h_all.rearrange("p (d t c b) -> p d t c b",
                                              d=2, t=L_eff, c=2)[
                            :, d, t0:t0 + TBLK, c, :].rearrange(
                            "p t b -> p b t")
                        nc.tensor.matmul(
                            out=eps[:],
                            lhsT=s_r(wlin_sb[:, kt * 33:(kt + 1) * 33]),
                            rhs=s_r(rhs), start=(kt == 0), stop=(kt == 3))
                    esb = emis_sb_p.tile([33, n], dt.float32r, tag="e")
                    nc.scalar.activation(out=esb[:], in_=eps[:],
                                         func=AF.Identity, bias=ebias_sb[:])
                    # CRF expand: tiles of 128 rows = 4 b's x TBLK t's
                    for rt in range(n // 128):
                        lhs = esb[:, rt * 128:(rt + 1) * 128]
                        nb = 128 // TBLK  # b's per row-tile
                        for hf in range(2):
                            cps = crf_ps.tile([128, 512], dt.float32, tag="c")
                            nc.tensor.matmul(
                                out=cps[:], lhsT=lhs,
                                rhs=jsel_sb[:, hf * 512:(hf + 1) * 512],
                                start=True, stop=True)
                            csb = crf_sb_p.tile([128, 512], dt.float32, tag="c")
                            if hf == 0:
                                nc.scalar.copy(out=csb[:], in_=cps[:])
                            else:
                                nc.vector.tensor_copy(out=csb[:], in_=cps[:])
                            dst = d_out[rt * nb:(rt + 1) * nb, t0:t0 + TBLK,
                                        hf * 512:(hf + 1) * 512]
                            nc.sync.dma_start(out=dst, in_=csb[:])

                TBLK = 32 if L_eff >= 64 else L_eff  # t's per output block
                NBLK = L_eff // TBLK

                for _rep in range(reps):
                    # middle-out emission schedule: after step s (0-indexed),
                    # t complete iff max(t, L-1-t) <= s.
                    emitted = [False] * NBLK
                    c_prev[0] = c_prev[1] = None
                    P_cur[0] = P_cur[1] = None

                    def maybe_emit(s):
                        for k in range(NBLK):
                            if emitted[k]:
                                continue
                            need = max((k + 1) * TBLK - 1,
                                       L_eff - 1 - k * TBLK)
                            if need <= s:
                                emitted[k] = True
                                emit_block(k)

                    for s in range(L_eff):
                        scan_step(0, s)
                        scan_step(1, s)
                        if s >= (L_eff // 2) and (s % 16 == 15
                                                  or s == L_eff - 1):
                            maybe_emit(s)
                    maybe_emit(L_eff - 1)
                    assert all(emitted)

                if timing:
                    tl = crf_sb_p.tile([1, 16], dt.float32, tag="tl")
                    nc.sync.dma_start(out=tl[:], in_=d_out[0, 0, 0:16])
                    nc.sync.dma_start(out=d_out_ext[:], in_=tl[:])

    nc.finalize()
    return nc


_CACHE = {}


def _get_nc(L_eff=L):
    if L_eff not in _CACHE:
        _CACHE[L_eff] = build_nc(L_eff)
    return _CACHE[L_eff]


def kernel(**inputs):
    from concourse.bass_utils import run_bass_kernel_spmd

    nc = _get_nc(L)
    in_maps = _host_prep(inputs, L)
    res = run_bass_kernel_spmd(nc, in_maps, list(range(NCORES)))
    outs = [res.results[c]["out"].reshape(BC, L, OUT, OUT)
            for c in range(NCORES)]
    return np.concatenate(outs, axis=0)


if __name__ == "__main__":
    nc = build_nc(64)
    print("built OK:", len(nc.m.functions[0].instructions)
          if hasattr(nc.m.functions[0], "instructions") else "?")



# revision 2
# speedup vs baseline: 3.4474x; 3.4474x over previous
"""BiLSTM-CRF Trainium2 kernel, v2 (direction-split, transposed gates).

Sharding: 8 cores = 4 sentence-groups x 2 directions. Core (d, g) runs
direction d (0=fwd, 1=bwd) of sentences [16g, 16g+16) and produces the
partial CRF tensor for its direction: fwd partial includes transition +
b_lin; bwd partial is just the bwd emission broadcast over i. kernel()
sums fwd+bwd partials per group (the unshard step) after flipping the
bwd core's time axis (bwd cores run in reversed "virtual time" v so the
program is SPMD-uniform).

Per-core device pipeline:
  1. Embedding gather (indirect DMA) in v-order, PE-transpose into
     xT [128=E, L*32] bf16 (32 cols per step: 16 real sentences + 16
     zero pads so per-step matmul output rows land on 32-partition
     boundaries).
  2. Scan, 4 steps per PSUM window [128, 1024]: window opens with
     x-side matmuls (xT_win.T @ WihT) + bias (ones.T @ bias row), then
     per step v: 4 accumulating matmuls add WhhT.T-side contribution
     for rows [32*(v%4), +32); sigmoid ACT over [32, 0:768] (i,f,o),
     tanh ACT over [32, 768:1024] (g), DVE cell update, h cast to bf16,
     2 transpose-DMAs store hT into h_all [128, (L+1)*64].
  3. Emission: per 128-token tile (8 v x 16 real b), 2 accumulating
     matmuls vs WlinT chunks -> eps [128, 32]; CRF = DVE broadcast add
     (eps repeated over i via stride-0 AP) + trans_rep; DMA out.
"""

import numpy as np

VOCAB, EMB, HID, OUT = 30000, 128, 256, 32
B, L = 64, 512
NCORES = 8
BC = 16           # sentences per core
BP = 32           # padded batch per step (16 real + 16 pad)
WIN = 4           # steps per PSUM window
USE_DMAT = True   # dma_start_transpose for hT


def _host_prep(inputs, L_eff=L):
    import ml_dtypes

    sents = np.asarray(inputs["sents_tensor"]).astype(np.int32)  # [B, L]
    emb = np.asarray(inputs["embedding"]).astype(np.float32)

    # torch gate order i,f,g,o (256 each) -> ours i,f,o,g
    perm = np.concatenate([np.arange(0, 512), np.arange(768, 1024),
                           np.arange(512, 768)])

    def dir_consts(Wih, Whh, bih, bhh, Wlin_half, trans_add):
        Wih = np.asarray(Wih, np.float32)[perm]      # [1024, 128]
        Whh = np.asarray(Whh, np.float32)[perm]      # [1024, 256]
        bias = (np.asarray(bih, np.float32) + np.asarray(bhh, np.float32))[perm]
        WhhT = np.ascontiguousarray(Whh.T)           # [256, 1024]
        WihT = np.ascontiguousarray(Wih.T)           # [128, 1024]
        WlinT = np.ascontiguousarray(Wlin_half.T)    # [256, 32]
        c2 = np.zeros([128, 4160], np.float32)
        for kt in range(2):
            c2[:, kt * 1024:(kt + 1) * 1024] = WhhT[kt * 128:(kt + 1) * 128]
        c2[:, 2048:3072] = WihT
        for kt in range(2):
            c2[:, 3072 + kt * 32:3072 + (kt + 1) * 32] = \
                WlinT[kt * 128:(kt + 1) * 128]
        c2[0, 3136:4160] = bias
        return c2.astype(ml_dtypes.bfloat16), trans_add.astype(np.float32)

    W_lin = np.asarray(inputs["W_lin"], np.float32)      # [32, 512]
    b_lin = np.asarray(inputs["b_lin"], np.float32)
    trans = np.asarray(inputs["transition"], np.float32)

    trans_f = np.broadcast_to(
        (trans + b_lin[None, :]).reshape(1, -1), (128, 1024)).copy()
    trans_b = np.zeros([128, 1024], np.float32)

    c2_f, tr_f = dir_consts(inputs["Wih_f"], inputs["Whh_f"], inputs["bih_f"],
                            inputs["bhh_f"], W_lin[:, 0:256], trans_f)
    c2_b, tr_b = dir_consts(inputs["Wih_b"], inputs["Whh_b"], inputs["bih_b"],
                            inputs["bhh_b"], W_lin[:, 256:512], trans_b)

    NG = L_eff * BC // 128  # gather tiles
    in_maps = []
    for core in range(NCORES):
        d, g = divmod(core, 4)
        S = sents[16 * g:16 * g + 16, :L_eff]   # [16, L]
        # v-order tokens: tok(v, b) = S[b, v] fwd / S[b, L-1-v] bwd
        Sv = S.T if d == 0 else S.T[::-1]       # [L, 16], row v
        idx = np.ascontiguousarray(
            Sv.reshape(NG, 128).T).astype(np.int32)  # [128, NG] p=8v'+... p = v_l*16+b
        c4 = np.zeros([128, NG + 1024], np.uint32)
        c4[:, 0:NG] = idx.view(np.uint32)
        c4[:, NG:NG + 1024] = (tr_f if d == 0 else tr_b).view(np.uint32)
        in_maps.append({
            "c4": np.ascontiguousarray(c4),
            "c2": np.ascontiguousarray(c2_f if d == 0 else c2_b),
            "emb": emb,
        })
    return in_maps


def build_nc(L_eff=L, reps=1, timing=False):
    import concourse.bass as bass
    import concourse.mybir as mybir
    import concourse.tile as tile
    from concourse.bacc import Bacc
    from concourse.masks import make_identity

    dt = mybir.dt
    AF = mybir.ActivationFunctionType
    OP = mybir.AluOpType

    NG = L_eff * BC // 128          # gather tiles
    NW = L_eff // WIN               # PSUM windows
    NE = L_eff // 4                 # emission tiles (4 v x 32 padded b)

    nc = Bacc()
    d_c4 = nc.declare_dram_parameter("c4", [128, NG + 1024], dt.uint32, False)
    d_c2 = nc.declare_dram_parameter("c2", [128, 4160], dt.bfloat16, False)
    if timing:
        d_emb = nc.dram_tensor("embt", [VOCAB, EMB], dt.float32)
        d_out = nc.dram_tensor("outt", [L_eff // 4, 128, 1024], dt.float32)
        d_out_ext = nc.declare_dram_parameter("out", [1, 16], dt.float32,
                                              isOutput=True)
    else:
        d_emb = nc.declare_dram_parameter("emb", [VOCAB, EMB], dt.float32,
                                          False)
        d_out = nc.declare_dram_parameter("out", [L_eff // 4, 128, 1024],
                                          dt.float32, isOutput=True)
        d_out_ext = None

    with tile.TileContext(nc) as tc:
        with (
            tc.tile_pool(name="const", bufs=1) as const,
            tc.tile_pool(name="state", bufs=1) as state,
            tc.tile_pool(name="gat", bufs=3) as gat,
            tc.tile_pool(name="misc_ps", bufs=2, space="PSUM") as misc_ps,
            tc.tile_pool(name="gw_ps", bufs=2, space="PSUM") as gw_ps,
            tc.tile_pool(name="tr_ps", bufs=2, space="PSUM") as tr_ps,
            tc.tile_pool(name="work", bufs=2) as work,
            tc.tile_pool(name="crf_sb", bufs=3) as crf_sb_p,
        ):
            ident = const.tile([128, 128], dt.float32)
            make_identity(nc, ident[:])
            ident_bf = const.tile([128, 128], dt.bfloat16)
            nc.vector.tensor_copy(out=ident_bf[:], in_=ident[:])
            ones1 = const.tile([1, 128], dt.bfloat16)
            nc.vector.memset(ones1[:], 1.0)
            c4_sb = const.tile([128, NG + 1024], dt.uint32)
            nc.sync.dma_start(out=c4_sb[:], in_=d_c4[:])
            idx_sb = c4_sb[:, 0:NG].bitcast(dt.int32)
            trans_rep = c4_sb[:, NG:NG + 1024].bitcast(dt.float32)
            c2_sb = const.tile([128, 4160], dt.bfloat16)
            nc.sync.dma_start(out=c2_sb[:], in_=d_c2[:])

            def whhT(kt, hf):  # [128, 512] moving operand
                return c2_sb[:, kt * 1024 + hf * 512:kt * 1024 + (hf + 1) * 512]

            def wihT(hf):
                return c2_sb[:, 2048 + hf * 512:2048 + (hf + 1) * 512]

            def wlinT(kt):
                return c2_sb[:, 3072 + kt * 32:3072 + (kt + 1) * 32]

            bias_row = c2_sb[0:1, 3136:4160]  # [1, 1024]

            # persistent buffers
            xT = state.tile([128, L_eff * BP], dt.bfloat16)
            # kt-major so emission reads one contiguous free dim per chunk
            h_all = state.tile([128, 2 * (L_eff + 1) * BP], dt.bfloat16)
            c_st = state.tile([BP, 256], dt.float32)
            nc.vector.memset(h_all[:, 0:BP], 0.0)
            nc.vector.memset(
                h_all[:, (L_eff + 1) * BP:(L_eff + 2) * BP], 0.0)

            def hslot(v, kt):  # stationary hT [128, 32] for step v
                off = (kt * (L_eff + 1) + v) * BP
                return h_all[:, off:off + BP]

            # ---- phase 1: gather + transpose into xT (outside reps) ----
            nc.vector.memset(xT[:], 0.0)
            for gi in range(0 if timing else NG):
                gt = gat.tile([128, 128], dt.float32, tag="g")
                nc.gpsimd.indirect_dma_start(
                    out=gt[:], out_offset=None, in_=d_emb[:],
                    in_offset=bass.IndirectOffsetOnAxis(
                        ap=idx_sb[:, gi:gi + 1], axis=0))
                tp = misc_ps.tile([128, 128], dt.float32, tag="m")
                nc.tensor.transpose(out=tp[:], in_=gt[:], identity=ident[:])
                # dst: cols v0*BP + v_l*BP + b  (v_l in 0..8, b in 0..16)
                dst = xT[:, gi * 8 * BP:(gi + 1) * 8 * BP].rearrange(
                    "p (v c) -> p v c", v=8)[:, :, 0:16]
                nc.vector.tensor_copy(
                    out=dst,
                    in_=tp.rearrange("p (v c) -> p v c", v=8))

            def scan_and_emit():
                for w in range(NW):
                    gps = gw_ps.tile([128, 1024], dt.float32, tag="gw")
                    for hf in range(2):
                        nc.tensor.matmul(
                            out=gps[:, hf * 512:(hf + 1) * 512],
                            lhsT=xT[:, w * 128:(w + 1) * 128],
                            rhs=wihT(hf), start=True, stop=False,
                            skip_group_check=True)
                        nc.tensor.matmul(
                            out=gps[:, hf * 512:(hf + 1) * 512],
                            lhsT=ones1[:], rhs=bias_row[:, hf * 512:
                                                        (hf + 1) * 512],
                            start=False, stop=False, skip_group_check=True)
                    for s in range(WIN):
                        v = w * WIN + s
                        r0 = s * BP
                        for hf in range(2):
                            for kt in range(2):
                                nc.tensor.matmul(
                                    out=gps[r0:r0 + BP,
                                            hf * 512:(hf + 1) * 512],
                                    lhsT=hslot(v, kt), rhs=whhT(kt, hf),
                                    start=False,
                                    stop=(s == WIN - 1 and hf == 1
                                          and kt == 1),
                                    skip_group_check=True,
                                    tile_position=(0, r0))
                        sig = work.tile([BP, 768], dt.float32, tag="sig")
                        nc.scalar.activation(out=sig[:],
                                             in_=gps[r0:r0 + BP, 0:768],
                                             func=AF.Sigmoid)
                        thg = work.tile([BP, 256], dt.float32, tag="thg")
                        nc.scalar.activation(out=thg[:],
                                             in_=gps[r0:r0 + BP, 768:1024],
                                             func=AF.Tanh)
                        a = work.tile([BP, 256], dt.float32, tag="a")
                        nc.vector.tensor_tensor(out=a[:], in0=sig[:, 0:256],
                                                in1=thg[:], op=OP.mult)
                        if v > 0:
                            bb = work.tile([BP, 256], dt.float32, tag="b")
                            nc.vector.tensor_tensor(out=bb[:],
                                                    in0=sig[:, 256:512],
                                                    in1=c_st[:], op=OP.mult)
                            nc.vector.tensor_tensor(out=c_st[:], in0=a[:],
                                                    in1=bb[:], op=OP.add)
                        else:
                            nc.vector.tensor_copy(out=c_st[:], in_=a[:])
                        th = work.tile([BP, 256], dt.float32, tag="th")
                        nc.scalar.activation(out=th[:], in_=c_st[:],
                                             func=AF.Tanh)
                        h = work.tile([BP, 256],
                                      dt.bfloat16 if USE_DMAT else dt.float32,
                                      tag="h")
                        nc.vector.tensor_tensor(out=h[:], in0=sig[:, 512:768],
                                                in1=th[:], op=OP.mult)
                        for kt in range(2):
                            if USE_DMAT:
                                nc.sync.dma_start_transpose(
                                    out=hslot(v + 1, kt),
                                    in_=h[:, kt * 128:(kt + 1) * 128])
                            else:
                                tps = tr_ps.tile([128, 32], dt.float32,
                                                 tag="tr")
                                nc.tensor.transpose(
                                    out=tps[:], in_=h[:, kt * 128:
                                                      (kt + 1) * 128],
                                    identity=ident[0:BP, 0:BP])
                                nc.vector.tensor_copy(out=hslot(v + 1, kt),
                                                      in_=tps[:])

                # ---- emission + CRF (4 v per tile, pad rows included) ----
                for e in range(NE):
                    v0 = e * 4
                    eps = misc_ps.tile([128, 32], dt.float32, tag="m")
                    for kt in range(2):
                        off = (kt * (L_eff + 1) + v0 + 1) * BP
                        nc.tensor.matmul(out=eps[:],
                                         lhsT=h_all[:, off:off + 4 * BP],
                                         rhs=wlinT(kt), start=(kt == 0),
                                         stop=(kt == 1))
                    crf = crf_sb_p.tile([128, 1024], dt.float32, tag="c")
                    eps_b = eps[:].rearrange("p (o j) -> p o j",
                                             o=1).broadcast_to((128, 32, 32))
                    nc.vector.tensor_tensor(
                        out=crf[:], in0=eps_b,
                        in1=trans_rep.rearrange("p (i j) -> p i j", i=32),
                        op=OP.add)
                    # full padded tile out; host strips pad rows
                    nc.sync.dma_start(out=d_out[e], in_=crf[:])

            for _rep in range(reps):
                scan_and_emit()

            if timing:
                tl = crf_sb_p.tile([1, 16], dt.float32, tag="tl")
                nc.sync.dma_start(out=tl[:], in_=d_out[0, 0, 0:16])
                nc.sync.dma_start(out=d_out_ext[:], in_=tl[:])

    nc.finalize()
    return nc


_CACHE = {}


def _get_nc(L_eff=L):
    if L_eff not in _CACHE:
        _CACHE[L_eff] = build_nc(L_eff)
    return _CACHE[L_eff]


def kernel(**inputs):
    from concourse.bass_utils import run_bass_kernel_spmd

    L_eff = np.asarray(inputs["sents_tensor"]).shape[1]
    nc = _get_nc(L_eff)
    in_maps = _host_prep(inputs, L_eff)
    res = run_bass_kernel_spmd(nc, in_maps, list(range(NCORES)))

    def unpack(o):
        # o: [L/4, 128, 1024]; rows = (v_l 4, b 32) with b<16 real
        o = o.reshape(L_eff // 4, 4, 32, 1024)[:, :, 0:16, :]
        return o.reshape(L_eff, BC, 1024).transpose(1, 0, 2)  # [BC, L, 1024]

    out = np.zeros([B, L_eff, OUT, OUT], np.float32)
    for g in range(4):
        f = unpack(res.results[g]["out"])
        bwd = unpack(res.results[4 + g]["out"])[:, ::-1]
        out[16 * g:16 * g + 16] = (f + bwd).reshape(BC, L_eff, OUT, OUT)
    return out


if __name__ == "__main__":
    nc = build_nc(64)
    print("built OK")


# revision 3
# speedup vs baseline: 3.6844x; 1.0688x over previous
"""BiLSTM-CRF Trainium2 kernel, v2 (direction-split, transposed gates).

Sharding: 8 cores = 4 sentence-groups x 2 directions. Core (d, g) runs
direction d (0=fwd, 1=bwd) of sentences [16g, 16g+16) and produces the
partial CRF tensor for its direction: fwd partial includes transition +
b_lin; bwd partial is just the bwd emission broadcast over i. kernel()
sums fwd+bwd partials per group (the unshard step) after flipping the
bwd core's time axis (bwd cores run in reversed "virtual time" v so the
program is SPMD-uniform).

Per-core device pipeline:
  1. Embedding gather (indirect DMA) in v-order, PE-transpose into
     xT [128=E, L*32] bf16 (32 cols per step: 16 real sentences + 16
     zero pads so per-step matmul output rows land on 32-partition
     boundaries).
  2. Scan, 4 steps per PSUM window [128, 1024]: window opens with
     x-side matmuls (xT_win.T @ WihT) + bias (ones.T @ bias row), then
     per step v: 4 accumulating matmuls add WhhT.T-side contribution
     for rows [32*(v%4), +32); sigmoid ACT over [32, 0:768] (i,f,o),
     tanh ACT over [32, 768:1024] (g), DVE cell update, h cast to bf16,
     2 transpose-DMAs store hT into h_all [128, (L+1)*64].
  3. Emission: per 128-token tile (8 v x 16 real b), 2 accumulating
     matmuls vs WlinT chunks -> eps [128, 32]; CRF = DVE broadcast add
     (eps repeated over i via stride-0 AP) + trans_rep; DMA out.
"""

import numpy as np

VOCAB, EMB, HID, OUT = 30000, 128, 256, 32
B, L = 64, 512
NCORES = 8
BC = 16           # sentences per core
BP = 32           # padded batch per step (16 real + 16 pad)
WIN = 4           # steps per PSUM window
USE_DMAT = True   # dma_start_transpose for hT


def _host_prep(inputs, L_eff=L):
    import ml_dtypes

    sents = np.asarray(inputs["sents_tensor"]).astype(np.int32)  # [B, L]
    emb = np.asarray(inputs["embedding"]).astype(np.float32)

    # torch gate order i,f,g,o (256 each) -> ours i,f,o,g
    perm = np.concatenate([np.arange(0, 512), np.arange(768, 1024),
                           np.arange(512, 768)])

    def dir_consts(Wih, Whh, bih, bhh, Wlin_half, trans_add):
        Wih = np.asarray(Wih, np.float32)[perm].copy()   # [1024, 128]
        Whh = np.asarray(Whh, np.float32)[perm].copy()   # [1024, 256]
        bias = (np.asarray(bih, np.float32)
                + np.asarray(bhh, np.float32))[perm].copy()
        # g block scaled by 2: tanh(g) computed as 2*sigmoid(2g)-1 so one
        # sigmoid ACT covers all four gates
        Wih[768:1024] *= 2.0
        Whh[768:1024] *= 2.0
        bias[768:1024] *= 2.0
        WhhT = np.ascontiguousarray(Whh.T)           # [256, 1024]
        WihT = np.ascontiguousarray(Wih.T)           # [128, 1024]
        WlinT = np.ascontiguousarray(Wlin_half.T)    # [256, 32]
        c2 = np.zeros([128, 4160], np.float32)
        for kt in range(2):
            c2[:, kt * 1024:(kt + 1) * 1024] = WhhT[kt * 128:(kt + 1) * 128]
        c2[:, 2048:3072] = WihT
        for kt in range(2):
            c2[:, 3072 + kt * 32:3072 + (kt + 1) * 32] = \
                WlinT[kt * 128:(kt + 1) * 128]
        c2[0, 3136:4160] = bias
        return c2.astype(ml_dtypes.bfloat16), trans_add.astype(np.float32)

    W_lin = np.asarray(inputs["W_lin"], np.float32)      # [32, 512]
    b_lin = np.asarray(inputs["b_lin"], np.float32)
    trans = np.asarray(inputs["transition"], np.float32)

    trans_f = np.broadcast_to(
        (trans + b_lin[None, :]).reshape(1, -1), (128, 1024)).copy()
    trans_b = np.zeros([128, 1024], np.float32)

    c2_f, tr_f = dir_consts(inputs["Wih_f"], inputs["Whh_f"], inputs["bih_f"],
                            inputs["bhh_f"], W_lin[:, 0:256], trans_f)
    c2_b, tr_b = dir_consts(inputs["Wih_b"], inputs["Whh_b"], inputs["bih_b"],
                            inputs["bhh_b"], W_lin[:, 256:512], trans_b)

    NG = L_eff * BC // 128  # gather tiles
    in_maps = []
    for core in range(NCORES):
        d, g = divmod(core, 4)
        S = sents[16 * g:16 * g + 16, :L_eff]   # [16, L]
        # v-order tokens: tok(v, b) = S[b, v] fwd / S[b, L-1-v] bwd
        Sv = S.T if d == 0 else S.T[::-1]       # [L, 16], row v
        idx = np.ascontiguousarray(
            Sv.reshape(NG, 128).T).astype(np.int32)  # [128, NG] p=8v'+... p = v_l*16+b
        c4 = np.zeros([128, NG + 1024], np.uint32)
        c4[:, 0:NG] = idx.view(np.uint32)
        c4[:, NG:NG + 1024] = (tr_f if d == 0 else tr_b).view(np.uint32)
        in_maps.append({
            "c4": np.ascontiguousarray(c4),
            "c2": np.ascontiguousarray(c2_f if d == 0 else c2_b),
            "emb": emb,
        })
    return in_maps


def build_nc(L_eff=L, reps=1, timing=False, with_bias=False):
    import concourse.bass as bass
    import concourse.mybir as mybir
    import concourse.tile as tile
    from concourse.bacc import Bacc
    from concourse.masks import make_identity

    dt = mybir.dt
    AF = mybir.ActivationFunctionType
    OP = mybir.AluOpType

    NG = L_eff * BC // 128          # gather tiles
    NW = L_eff // WIN               # PSUM windows
    NE = L_eff // 4                 # emission tiles (4 v x 32 padded b)

    nc = Bacc()
    d_c4 = nc.declare_dram_parameter("c4", [128, NG + 1024], dt.uint32, False)
    d_c2 = nc.declare_dram_parameter("c2", [128, 4160], dt.bfloat16, False)
    if timing:
        d_emb = nc.dram_tensor("embt", [VOCAB, EMB], dt.float32)
        d_out = nc.dram_tensor("outt", [L_eff // 4, 128, 1024], dt.float32)
        d_out_ext = nc.declare_dram_parameter("out", [1, 16], dt.float32,
                                              isOutput=True)
    else:
        d_emb = nc.declare_dram_parameter("emb", [VOCAB, EMB], dt.float32,
                                          False)
        d_out = nc.declare_dram_parameter("out", [L_eff // 4, 128, 1024],
                                          dt.float32, isOutput=True)
        d_out_ext = None

    with tile.TileContext(nc) as tc:
        with (
            tc.tile_pool(name="const", bufs=1) as const,
            tc.tile_pool(name="state", bufs=1) as state,
            tc.tile_pool(name="gat", bufs=3) as gat,
            tc.tile_pool(name="misc_ps", bufs=2, space="PSUM") as misc_ps,
            tc.tile_pool(name="gw_ps", bufs=2, space="PSUM") as gw_ps,
            tc.tile_pool(name="tr_ps", bufs=2, space="PSUM") as tr_ps,
            tc.tile_pool(name="work", bufs=2) as work,
            tc.tile_pool(name="crf_sb", bufs=3) as crf_sb_p,
        ):
            ident = const.tile([128, 128], dt.float32)
            make_identity(nc, ident[:])
            ident_bf = const.tile([128, 128], dt.bfloat16)
            nc.vector.tensor_copy(out=ident_bf[:], in_=ident[:])
            ones1 = const.tile([1, 128], dt.bfloat16)
            nc.vector.memset(ones1[:], 1.0)
            c4_sb = const.tile([128, NG + 1024], dt.uint32)
            nc.sync.dma_start(out=c4_sb[:], in_=d_c4[:])
            idx_sb = c4_sb[:, 0:NG].bitcast(dt.int32)
            trans_rep = c4_sb[:, NG:NG + 1024].bitcast(dt.float32)
            c2_sb = const.tile([128, 4160], dt.bfloat16)
            nc.sync.dma_start(out=c2_sb[:], in_=d_c2[:])

            def whhT(kt, hf):  # [128, 512] moving operand
                return c2_sb[:, kt * 1024 + hf * 512:kt * 1024 + (hf + 1) * 512]

            def wihT(hf):
                return c2_sb[:, 2048 + hf * 512:2048 + (hf + 1) * 512]

            def wlinT(kt):
                return c2_sb[:, 3072 + kt * 32:3072 + (kt + 1) * 32]

            bias_row = c2_sb[0:1, 3136:4160]  # [1, 1024]

            # persistent buffers
            xT = state.tile([128, L_eff * BP], dt.bfloat16)
            # kt-major so emission reads one contiguous free dim per chunk
            h_all = state.tile([128, 2 * (L_eff + 1) * BP], dt.bfloat16)
            c_st = state.tile([BP, 256], dt.float32)
            nc.vector.memset(h_all[:, 0:BP], 0.0)
            nc.vector.memset(
                h_all[:, (L_eff + 1) * BP:(L_eff + 2) * BP], 0.0)

            def hslot(v, kt):  # stationary hT [128, 32] for step v
                off = (kt * (L_eff + 1) + v) * BP
                return h_all[:, off:off + BP]

            # ---- phase 1: gather + transpose into xT (outside reps) ----
            nc.vector.memset(xT[:], 0.0)
            for gi in range(0 if timing else NG):
                gt = gat.tile([128, 128], dt.float32, tag="g")
                nc.gpsimd.indirect_dma_start(
                    out=gt[:], out_offset=None, in_=d_emb[:],
                    in_offset=bass.IndirectOffsetOnAxis(
                        ap=idx_sb[:, gi:gi + 1], axis=0))
                tp = misc_ps.tile([128, 128], dt.float32, tag="m")
                nc.tensor.transpose(out=tp[:], in_=gt[:], identity=ident[:])
                # dst: cols v0*BP + v_l*BP + b  (v_l in 0..8, b in 0..16)
                dst = xT[:, gi * 8 * BP:(gi + 1) * 8 * BP].rearrange(
                    "p (v c) -> p v c", v=8)[:, :, 0:16]
                nc.vector.tensor_copy(
                    out=dst,
                    in_=tp.rearrange("p (v c) -> p v c", v=8))

            def scan_and_emit():
                for w in range(NW):
                    gps = gw_ps.tile([128, 1024], dt.float32, tag="gw")
                    for hf in range(2):
                        nc.tensor.matmul(
                            out=gps[:, hf * 512:(hf + 1) * 512],
                            lhsT=xT[:, w * 128:(w + 1) * 128],
                            rhs=wihT(hf), start=True, stop=False,
                            skip_group_check=True)
                        if with_bias:
                            nc.tensor.matmul(
                                out=gps[:, hf * 512:(hf + 1) * 512],
                                lhsT=ones1[:], rhs=bias_row[:, hf * 512:
                                                            (hf + 1) * 512],
                                start=False, stop=False,
                                skip_group_check=True)
                    for s in range(WIN):
                        v = w * WIN + s
                        r0 = s * BP
                        for hf in range(2):
                            for kt in range(2):
                                nc.tensor.matmul(
                                    out=gps[r0:r0 + BP,
                                            hf * 512:(hf + 1) * 512],
                                    lhsT=hslot(v, kt), rhs=whhT(kt, hf),
                                    start=False,
                                    stop=(s == WIN - 1 and hf == 1
                                          and kt == 1),
                                    skip_group_check=True,
                                    tile_position=(0, r0))
                        sig = work.tile([BP, 1024], dt.float32, tag="sig")
                        nc.scalar.activation(out=sig[:],
                                             in_=gps[r0:r0 + BP, 0:1024],
                                             func=AF.Sigmoid)
                        thg = work.tile([BP, 256], dt.float32, tag="thg")
                        nc.vector.tensor_scalar(out=thg[:],
                                                in0=sig[:, 768:1024],
                                                scalar1=2.0, scalar2=-1.0,
                                                op0=OP.mult, op1=OP.add)
                        a = work.tile([BP, 256], dt.float32, tag="a")
                        nc.vector.tensor_tensor(out=a[:], in0=sig[:, 0:256],
                                                in1=thg[:], op=OP.mult)
                        if v > 0:
                            bb = work.tile([BP, 256], dt.float32, tag="b")
                            nc.vector.tensor_tensor(out=bb[:],
                                                    in0=sig[:, 256:512],
                                                    in1=c_st[:], op=OP.mult)
                            nc.vector.tensor_tensor(out=c_st[:], in0=a[:],
                                                    in1=bb[:], op=OP.add)
                        else:
                            nc.vector.tensor_copy(out=c_st[:], in_=a[:])
                        th = work.tile([BP, 256], dt.float32, tag="th")
                        nc.scalar.activation(out=th[:], in_=c_st[:],
                                             func=AF.Tanh)
                        h = work.tile([BP, 256],
                                      dt.bfloat16 if USE_DMAT else dt.float32,
                                      tag="h")
                        nc.vector.tensor_tensor(out=h[:], in0=sig[:, 512:768],
                                                in1=th[:], op=OP.mult)
                        for kt in range(2):
                            if USE_DMAT:
                                nc.sync.dma_start_transpose(
                                    out=hslot(v + 1, kt),
                                    in_=h[:, kt * 128:(kt + 1) * 128])
                            else:
                                tps = tr_ps.tile([128, 32], dt.float32,
                                                 tag="tr")
                                nc.tensor.transpose(
                                    out=tps[:], in_=h[:, kt * 128:
                                                      (kt + 1) * 128],
                                    identity=ident[0:BP, 0:BP])
                                nc.vector.tensor_copy(out=hslot(v + 1, kt),
                                                      in_=tps[:])

                # ---- emission + CRF (4 v per tile, pad rows included) ----
                for e in range(NE):
                    v0 = e * 4
                    eps = misc_ps.tile([128, 32], dt.float32, tag="m")
                    for kt in range(2):
                        off = (kt * (L_eff + 1) + v0 + 1) * BP
                        nc.tensor.matmul(out=eps[:],
                                         lhsT=h_all[:, off:off + 4 * BP],
                                         rhs=wlinT(kt), start=(kt == 0),
                                         stop=(kt == 1))
                    crf = crf_sb_p.tile([128, 1024], dt.float32, tag="c")
                    eps_b = eps[:].rearrange("p (o j) -> p o j",
                                             o=1).broadcast_to((128, 32, 32))
                    nc.vector.tensor_tensor(
                        out=crf[:], in0=eps_b,
                        in1=trans_rep.rearrange("p (i j) -> p i j", i=32),
                        op=OP.add)
                    # full padded tile out; host strips pad rows
                    nc.sync.dma_start(out=d_out[e], in_=crf[:])

            for _rep in range(reps):
                scan_and_emit()

            if timing:
                tl = crf_sb_p.tile([1, 16], dt.float32, tag="tl")
                nc.sync.dma_start(out=tl[:], in_=d_out[0, 0, 0:16])
                nc.sync.dma_start(out=d_out_ext[:], in_=tl[:])

    nc.finalize()
    return nc


_CACHE = {}


def _get_nc(L_eff=L, with_bias=False):
    key = (L_eff, with_bias)
    if key not in _CACHE:
        _CACHE[key] = build_nc(L_eff, with_bias=with_bias)
    return _CACHE[key]


def kernel(**inputs):
    from concourse.bass_utils import run_bass_kernel_spmd

    L_eff = np.asarray(inputs["sents_tensor"]).shape[1]
    with_bias = any(
        np.any(np.asarray(inputs[k])) for k in
        ("bih_f", "bhh_f", "bih_b", "bhh_b"))
    nc = _get_nc(L_eff, with_bias)
    in_maps = _host_prep(inputs, L_eff)
    res = run_bass_kernel_spmd(nc, in_maps, list(range(NCORES)))

    def unpack(o):
        # o: [L/4, 128, 1024]; rows = (v_l 4, b 32) with b<16 real
        o = o.reshape(L_eff // 4, 4, 32, 1024)[:, :, 0:16, :]
        return o.reshape(L_eff, BC, 1024).transpose(1, 0, 2)  # [BC, L, 1024]

    out = np.zeros([B, L_eff, OUT, OUT], np.float32)
    for g in range(4):
        f = unpack(res.results[g]["out"])
        bwd = unpack(res.results[4 + g]["out"])[:, ::-1]
        out[16 * g:16 * g + 16] = (f + bwd).reshape(BC, L_eff, OUT, OUT)
    return out


if __name__ == "__main__":
    nc = build_nc(64)
    print("built OK")
